# revision 2
# baseline (speedup 1.0000x reference)
"""3D Haar DWT (nn_Patcher) Trainium2 Bass kernel.

Math: with even dims and reflect-pad(0,1) never engaged, the reference is
non-overlapping 2x2x2 Haar butterflies; r^3 * 2*sqrt(2) == 1 exactly, so the
output is pure +/-1 sums over 2x2x2 blocks -- no multiplies needed.

Full input  x  [4, 3, 32, 256, 256] f32
Full output y  [4, 24, 16, 128, 128] f32   (8 subbands x 3 ch on channel dim)

Sharding (8 cores): core k -> (b = k//2, th = k%2); shard input
x[b, :, 16*th : 16*th+16]  -> [3, 16, 256, 256]  (12.58 MB)
shard output -> y[b, :, 8*th : 8*th+8]           -> [24, 8, 128, 128]

Per-core kernel: 6 mega-iters over (c in 3, tg in 2). Partition p = t*32 + h4
(t = output frame in mega-iter 0..3, h4 = h_out//4 0..31). All three Haar
stages stay within-partition:
  T-stage: lo on DVE, hi on Pool      (frame parity, contiguous FD=2048)
  H-stage: lo on DVE, hi on Pool      (row parity)
  W-stage: hi on Pool; lo alternates DVE/Pool per iter (engine balance)
DMA: each engine's HWDGE ring serializes its transfers, so input DMAs
alternate SP/Act and output DMAs alternate Act/SP to use both rings and
let outputs start as soon as each mega-iter finishes.
"""

import sys

for _p in ("/opt/trn_rl_repo", "/opt/pypackages"):
    if _p not in sys.path:
        sys.path.append(_p)

import numpy as np

_NC_CACHE = {}


def _build(reps=1):
    if reps in _NC_CACHE:
        return _NC_CACHE[reps]

    from concourse import bacc, mybir
    from concourse.tile import TileContext

    fp32 = mybir.dt.float32
    add = mybir.AluOpType.add
    sub = mybir.AluOpType.subtract

    # Bacc (not plain Bass): its finalize() runs the backend passes, incl.
    # generate_event_semaphores which splits multi-wait instructions (HW
    # allows at most 1 sync wait per instruction).
    nc = bacc.Bacc(None, target_bir_lowering=False)
    x = nc.dram_tensor("x_shard", [3, 16, 256, 256], fp32, kind="ExternalInput")
    y = nc.dram_tensor("y_shard", [24, 8, 128, 128], fp32, kind="ExternalOutput")

    # y viewed as [c, tg, (t h4), s, (hr w)] ; channels 24 = (s c), s=4tb+2hb+wb.
    # (t h4) merges to one stride-512 dim matching SBUF partitions; (hr w) is a
    # contiguous 2 KB run. 3-dim DMA AP both sides, partition-first on SBUF.
    yv = y[:].rearrange(
        "(s c) (tg t) (h4 hr) w -> c tg (t h4) s (hr w)", s=8, c=3, tg=2, hr=4
    )

    in_cycle = [nc.sync, nc.scalar]
    out_cycle = [nc.scalar, nc.sync]
    ni = 0
    no = 0

    with TileContext(nc) as tc:
        with tc.tile_pool(name="pool", bufs=2) as pool:
            it = 0
            for _rep in range(reps):
                for c in range(3):
                    for tg in range(2):
                        # tiles: [partition=128, ...free dims...], 2 MB each
                        t_in = pool.tile([128, 2, 4, 2, 256], fp32)  # (f, q, rp, w)
                        t_t = pool.tile([128, 2, 4, 2, 256], fp32)   # (tb, q, rp, w)
                        t_h = pool.tile([128, 2, 2, 4, 256], fp32)   # (tb, hb, q, w)
                        t_o = pool.tile([128, 2, 2, 2, 4, 128], fp32)  # (tb,hb,wb,q,w)

                        # ---- input DMA: 2 transfers of 1 MB (8 KB runs),
                        # split by frame parity so partition (t h4) merges ----
                        f0 = 8 * tg
                        for f in range(2):
                            src = x[c, f0 + f : f0 + 8 : 2].rearrange(
                                "t (h4 r) w -> t h4 (r w)", h4=32
                            )
                            dst = t_in[:, f].rearrange("p q r w -> p (q r w)")
                            in_cycle[ni % 2].dma_start(out=dst, in_=src)
                            ni += 1

                        V = nc.vector
                        P = nc.gpsimd

                        # ---- T stage (frame parity) ----
                        V.tensor_tensor(
                            out=t_t[:, 0], in0=t_in[:, 0], in1=t_in[:, 1], op=add
                        )
                        P.tensor_tensor(
                            out=t_t[:, 1], in0=t_in[:, 0], in1=t_in[:, 1], op=sub
                        )

                        # ---- H stage (row parity rp) ----
                        V.tensor_tensor(
                            out=t_h[:, :, 0],
                            in0=t_t[:, :, :, 0],
                            in1=t_t[:, :, :, 1],
                            op=add,
                        )
                        P.tensor_tensor(
                            out=t_h[:, :, 1],
                            in0=t_t[:, :, :, 0],
                            in1=t_t[:, :, :, 1],
                            op=sub,
                        )

                        # ---- W stage (column parity) ----
                        t_hv = t_h.rearrange(
                            "p a b q (wh wl) -> p a b q wh wl", wl=2
                        )
                        w0e = V if it % 2 == 0 else P
                        w0e.tensor_tensor(
                            out=t_o[:, :, :, 0],
                            in0=t_hv[:, :, :, :, :, 0],
                            in1=t_hv[:, :, :, :, :, 1],
                            op=add,
                        )
                        P.tensor_tensor(
                            out=t_o[:, :, :, 1],
                            in0=t_hv[:, :, :, :, :, 0],
                            in1=t_hv[:, :, :, :, :, 1],
                            op=sub,
                        )

                        # ---- output DMA: 1 transfer of 2 MB (2 KB runs) ----
                        src = t_o.rearrange("p a b v q w -> p (a b v) (q w)")
                        out_cycle[no % 2].dma_start(out=yv[c, tg], in_=src)
                        no += 1
                        it += 1

    nc.finalize()
    _NC_CACHE[reps] = nc
    return nc


def _in_maps(x):
    x = np.ascontiguousarray(np.asarray(x, dtype=np.float32))
    assert x.shape == (4, 3, 32, 256, 256), x.shape
    in_maps = []
    for k in range(8):
        b, th = divmod(k, 2)
        in_maps.append(
            {"x_shard": np.ascontiguousarray(x[b, :, 16 * th : 16 * th + 16])}
        )
    return in_maps


def _run(x, trace=False, **spmd_kwargs):
    from concourse.bass_utils import run_bass_kernel_spmd

    nc = _build()
    in_maps = _in_maps(x)

    bkr = run_bass_kernel_spmd(nc, in_maps, list(range(8)), trace=trace, **spmd_kwargs)

    out = np.empty((4, 24, 16, 128, 128), dtype=np.float32)
    for k in range(8):
        b, th = divmod(k, 2)
        out[b, :, 8 * th : 8 * th + 8] = np.asarray(bkr.results[k]["y_shard"])
    return out, bkr


def kernel(x):
    out, _ = _run(x)
    return out



# revision 3
# speedup vs baseline: 151.5159x; 151.5159x over previous
"""3D Haar DWT (nn_Patcher) Trainium2 Bass kernel.

Math: with even dims and reflect-pad(0,1) never engaged, the reference is
non-overlapping 2x2x2 Haar butterflies; r^3 * 2*sqrt(2) == 1 exactly, so the
output is pure +/-1 sums over 2x2x2 blocks -- no multiplies needed.

Full input  x  [4, 3, 32, 256, 256] f32
Full output y  [4, 24, 16, 128, 128] f32   (8 subbands x 3 ch on channel dim)

Sharding (8 cores): core k -> (b = k//2, th = k%2); shard input
x[b, :, 16*th : 16*th+16]  -> [3, 16, 256, 256]  (12.58 MB)
shard output -> y[b, :, 8*th : 8*th+8]           -> [24, 8, 128, 128]

Per-core kernel: 6 mega-iters over (c in 3, tg in 2). Partition p = t*32 + h4
(t = output frame in mega-iter 0..3, h4 = h_out//4 0..31). All three Haar
stages stay within-partition. Two deliberate perf choices vs the naive form:

* Output is written in bf16 (error ~1e-3 << the 2e-2 gate) which halves
  output HBM traffic: 12.58+6.29 MB/core = 18.87 MB -> 52.4 us DMA floor
  at 360 GB/s, vs 69.9 us for f32 out.
* Stage order is W -> T -> H, not T -> H -> W. The W butterfly reads
  stride-2 element pairs, which disqualifies the DVE 2x 16-bit mode, so W
  is done FIRST while the data is still f32 (which never gets 2x anyway)
  and converts to bf16 on write. T and H then read packed bf16 at 2x.
  DVE carries 5 of 6 ops (~6.7 us/iter), Pool (gpsimd, 0.42-efficiency
  software adds) carries only W-sub (~4.2 us/iter); both sit under the
  ~8.7 us/iter DMA floor, so steady state is DMA-bound.

DMA: each engine's HWDGE ring serializes its transfers, so input DMAs
alternate SP/Act and output DMAs alternate Act/SP to use both rings and
let outputs start as soon as each mega-iter finishes.
"""

import sys

for _p in ("/opt/trn_rl_repo", "/opt/pypackages"):
    if _p not in sys.path:
        sys.path.append(_p)

import numpy as np

_NC_CACHE = {}


def _build(reps=1):
    if reps in _NC_CACHE:
        return _NC_CACHE[reps]

    from concourse import bacc, mybir
    from concourse.tile import TileContext

    fp32 = mybir.dt.float32
    bf16 = mybir.dt.bfloat16
    add = mybir.AluOpType.add
    sub = mybir.AluOpType.subtract

    # Bacc (not plain Bass): its finalize() runs the backend passes, incl.
    # generate_event_semaphores which splits multi-wait instructions (HW
    # allows at most 1 sync wait per instruction).
    nc = bacc.Bacc(None, target_bir_lowering=False)
    x = nc.dram_tensor("x_shard", [3, 16, 256, 256], fp32, kind="ExternalInput")
    y = nc.dram_tensor("y_shard", [24, 8, 128, 128], bf16, kind="ExternalOutput")

    # y viewed as [c, tg, (t h4), s, (rh wh)] ; channels 24 = (s c),
    # s = 4*tb + 2*hb + wb. (t h4) merges to one stride-512 dim matching SBUF
    # partitions; (rh wh) is a contiguous 1 KB run. 3-dim DMA AP both sides.
    yv = y[:].rearrange(
        "(s c) (tg t) (h4 rh) wh -> c tg (t h4) s (rh wh)", s=8, c=3, tg=2, rh=4
    )

    in_cycle = [nc.sync, nc.scalar]
    out_cycle = [nc.scalar, nc.sync]
    ni = 0
    no = 0

    with TileContext(nc) as tc:
        with tc.tile_pool(name="pool", bufs=2) as pool:
            for _rep in range(reps):
                for c in range(3):
                    for tg in range(2):
                        # partition p = (t, h4); free dims per tile:
                        # t_in: (f, r, w)          f=frame parity, r=row in blk
                        # t_w:  (f, r, wb, wh)     wb=w-band (lo/hi)
                        # t_t:  (tb, r, wb, wh)    tb=t-band
                        # t_o:  (tb, hb, wb, rh, wh)
                        t_in = pool.tile([128, 2, 8, 256], fp32)
                        t_w = pool.tile([128, 2, 8, 2, 128], bf16)
                        t_t = pool.tile([128, 2, 8, 2, 128], bf16)
                        t_o = pool.tile([128, 2, 2, 2, 4, 128], bf16)

                        # ---- input DMA: 2 transfers of 1 MB (8 KB runs),
                        # split by frame parity so partition (t h4) merges ----
                        f0 = 8 * tg
                        for f in range(2):
                            src = x[c, f0 + f : f0 + 8 : 2].rearrange(
                                "t (h4 r) w -> t h4 (r w)", h4=32
                            )
                            dst = t_in[:, f].rearrange("p r w -> p (r w)")
                            in_cycle[ni % 2].dma_start(out=dst, in_=src)
                            ni += 1

                        V = nc.vector
                        P = nc.gpsimd

                        # ---- W stage (column parity; f32 in, bf16 out) ----
                        t_inv = t_in.rearrange(
                            "p f r (wh wl) -> p f r wh wl", wl=2
                        )
                        V.tensor_tensor(
                            out=t_w[:, :, :, 0],
                            in0=t_inv[:, :, :, :, 0],
                            in1=t_inv[:, :, :, :, 1],
                            op=add,
                        )
                        P.tensor_tensor(
                            out=t_w[:, :, :, 1],
                            in0=t_inv[:, :, :, :, 0],
                            in1=t_inv[:, :, :, :, 1],
                            op=sub,
                        )

                        # ---- T stage (frame parity; bf16 2x on DVE) ----
                        V.tensor_tensor(
                            out=t_t[:, 0], in0=t_w[:, 0], in1=t_w[:, 1], op=add
                        )
                        V.tensor_tensor(
                            out=t_t[:, 1], in0=t_w[:, 0], in1=t_w[:, 1], op=sub
                        )

                        # ---- H stage (row parity; bf16 2x on DVE) ----
                        t_tv = t_t.rearrange(
                            "p tb (rh rl) wb wh -> p tb rh rl wb wh", rl=2
                        )
                        for hb, op in ((0, add), (1, sub)):
                            V.tensor_tensor(
                                out=t_o[:, :, hb].rearrange(
                                    "p tb wb rh wh -> p tb rh wb wh"
                                ),
                                in0=t_tv[:, :, :, 0],
                                in1=t_tv[:, :, :, 1],
                                op=op,
                            )

                        # ---- output DMA: 1 transfer of 1 MB (1 KB runs) ----
                        src = t_o.rearrange("p tb hb wb rh wh -> p (tb hb wb) (rh wh)")
                        out_cycle[no % 2].dma_start(out=yv[c, tg], in_=src)
                        no += 1

    nc.finalize()
    _NC_CACHE[reps] = nc
    return nc


def _in_maps(x):
    x = np.ascontiguousarray(np.asarray(x, dtype=np.float32))
    assert x.shape == (4, 3, 32, 256, 256), x.shape
    in_maps = []
    for k in range(8):
        b, th = divmod(k, 2)
        in_maps.append(
            {"x_shard": np.ascontiguousarray(x[b, :, 16 * th : 16 * th + 16])}
        )
    return in_maps


def _run(x, trace=False, **spmd_kwargs):
    from concourse.bass_utils import run_bass_kernel_spmd

    nc = _build()
    in_maps = _in_maps(x)

    bkr = run_bass_kernel_spmd(nc, in_maps, list(range(8)), trace=trace, **spmd_kwargs)

    out = np.empty((4, 24, 16, 128, 128), dtype=np.float32)
    for k in range(8):
        b, th = divmod(k, 2)
        out[b, :, 8 * th : 8 * th + 8] = np.asarray(
            bkr.results[k]["y_shard"]
        ).astype(np.float32)
    return out, bkr


def kernel(x):
    out, _ = _run(x)
    return out


# revision 4
# speedup vs baseline: 159.3590x; 1.0518x over previous
"""3D Haar DWT (nn_Patcher) Trainium2 Bass kernel.

Math: with even dims and reflect-pad(0,1) never engaged, the reference is
non-overlapping 2x2x2 Haar butterflies; r^3 * 2*sqrt(2) == 1 exactly, so the
output is pure +/-1 sums over 2x2x2 blocks -- no multiplies needed.

Full input  x  [4, 3, 32, 256, 256] f32
Full output y  [4, 24, 16, 128, 128] f32   (8 subbands x 3 ch on channel dim)

Sharding (8 cores): core k -> (b = k//2, th = k%2); shard input
x[b, :, 16*th : 16*th+16]  -> [3, 16, 256, 256]  (12.58 MB)
shard output -> y[b, :, 8*th : 8*th+8]           -> [24, 8, 128, 128]

Per-core kernel: 6 mega-iters over (c in 3, tg in 2). Partition p = t*32 + h4
(t = output frame in mega-iter 0..3, h4 = h_out//4 0..31). All three Haar
stages stay within-partition. Two deliberate perf choices vs the naive form:

* Output is written in bf16 (error ~1e-3 << the 2e-2 gate) which halves
  output HBM traffic: 12.58+6.29 MB/core = 18.87 MB -> 52.4 us DMA floor
  at 360 GB/s, vs 69.9 us for f32 out.
* Stage order is W -> T -> H, not T -> H -> W. The W butterfly reads
  stride-2 element pairs, which disqualifies the DVE 2x 16-bit mode, so W
  is done FIRST while the data is still f32 (which never gets 2x anyway)
  and converts to bf16 on write. T and H then read packed bf16 at 2x.
  DVE carries 5 of 6 ops (~6.7 us/iter), Pool (gpsimd, 0.42-efficiency
  software adds) carries only W-sub (~4.2 us/iter); both sit under the
  ~8.7 us/iter DMA floor, so steady state is DMA-bound.

DMA: each engine's HWDGE ring serializes its transfers, so input DMAs
alternate SP/Act and output DMAs alternate Act/SP to use both rings and
let outputs start as soon as each mega-iter finishes.
"""

import sys

for _p in ("/opt/trn_rl_repo", "/opt/pypackages"):
    if _p not in sys.path:
        sys.path.append(_p)

import numpy as np

_NC_CACHE = {}


def _build(reps=1):
    if reps in _NC_CACHE:
        return _NC_CACHE[reps]

    from concourse import bacc, mybir
    from concourse.tile import TileContext

    fp32 = mybir.dt.float32
    bf16 = mybir.dt.bfloat16
    add = mybir.AluOpType.add
    sub = mybir.AluOpType.subtract

    # Bacc (not plain Bass): its finalize() runs the backend passes, incl.
    # generate_event_semaphores which splits multi-wait instructions (HW
    # allows at most 1 sync wait per instruction).
    nc = bacc.Bacc(None, target_bir_lowering=False)
    x = nc.dram_tensor("x_shard", [3, 16, 256, 256], fp32, kind="ExternalInput")
    y = nc.dram_tensor("y_shard", [24, 8, 128, 128], bf16, kind="ExternalOutput")

    # y viewed as [c, tg, (t h4), s, (rh wh)] ; channels 24 = (s c),
    # s = 4*tb + 2*hb + wb. (t h4) merges to one stride-512 dim matching SBUF
    # partitions; (rh wh) is a contiguous 1 KB run. 3-dim DMA AP both sides.
    yv = y[:].rearrange(
        "(s c) (tg t) (h4 rh) wh -> c tg (t h4) s (rh wh)", s=8, c=3, tg=2, rh=4
    )

    in_cycle = [nc.sync, nc.scalar]
    out_cycle = [nc.scalar, nc.sync]
    ni = 0
    no = 0

    with TileContext(nc) as tc:
        with tc.tile_pool(name="pool", bufs=2) as pool:
            for _rep in range(reps):
                for c in range(3):
                    for tg in range(2):
                        # partition p = (t, h4); free dims per tile:
                        # t_in: (f, r, w)          f=frame parity, r=row in blk
                        # t_w:  (f, r, wb, wh)     wb=w-band (lo/hi)
                        # t_t:  (tb, r, wb, wh)    tb=t-band
                        # t_o:  (tb, hb, wb, rh, wh)
                        t_in = pool.tile([128, 2, 8, 256], fp32)
                        t_w = pool.tile([128, 2, 8, 2, 128], bf16)
                        t_t = pool.tile([128, 2, 8, 2, 128], bf16)
                        t_o = pool.tile([128, 2, 2, 2, 4, 128], bf16)

                        # ---- input DMA: 2 transfers of 1 MB (8 KB runs),
                        # split by frame parity so partition (t h4) merges ----
                        f0 = 8 * tg
                        for f in range(2):
                            src = x[c, f0 + f : f0 + 8 : 2].rearrange(
                                "t (h4 r) w -> t h4 (r w)", h4=32
                            )
                            dst = t_in[:, f].rearrange("p r w -> p (r w)")
                            in_cycle[ni % 2].dma_start(out=dst, in_=src)
                            ni += 1

                        V = nc.vector
                        P = nc.gpsimd

                        # ---- W stage (column parity; f32 in, bf16 out) ----
                        t_inv = t_in.rearrange(
                            "p f r (wh wl) -> p f r wh wl", wl=2
                        )
                        V.tensor_tensor(
                            out=t_w[:, :, :, 0],
                            in0=t_inv[:, :, :, :, 0],
                            in1=t_inv[:, :, :, :, 1],
                            op=add,
                        )
                        P.tensor_tensor(
                            out=t_w[:, :, :, 1],
                            in0=t_inv[:, :, :, :, 0],
                            in1=t_inv[:, :, :, :, 1],
                            op=sub,
                        )

                        # ---- T stage (frame parity; bf16 2x on DVE) ----
                        V.tensor_tensor(
                            out=t_t[:, 0], in0=t_w[:, 0], in1=t_w[:, 1], op=add
                        )
                        V.tensor_tensor(
                            out=t_t[:, 1], in0=t_w[:, 0], in1=t_w[:, 1], op=sub
                        )

                        # ---- H stage (row parity; bf16 2x on DVE) ----
                        # 4 ops (tb x hb): ISA mem patterns allow at most 3
                        # free dims per operand, so tb can't be a 4th dim.
                        t_tv = t_t.rearrange(
                            "p tb (rh rl) wb wh -> p tb rh rl wb wh", rl=2
                        )
                        for tb in range(2):
                            for hb, op in ((0, add), (1, sub)):
                                V.tensor_tensor(
                                    out=t_o[:, tb, hb],
                                    in0=t_tv[:, tb, :, 0].rearrange(
                                        "p rh wb wh -> p wb rh wh"
                                    ),
                                    in1=t_tv[:, tb, :, 1].rearrange(
                                        "p rh wb wh -> p wb rh wh"
                                    ),
                                    op=op,
                                )

                        # ---- output DMA: 1 transfer of 1 MB (1 KB runs) ----
                        src = t_o.rearrange("p tb hb wb rh wh -> p (tb hb wb) (rh wh)")
                        out_cycle[no % 2].dma_start(out=yv[c, tg], in_=src)
                        no += 1

    nc.finalize()
    _NC_CACHE[reps] = nc
    return nc


def _in_maps(x):
    x = np.ascontiguousarray(np.asarray(x, dtype=np.float32))
    assert x.shape == (4, 3, 32, 256, 256), x.shape
    in_maps = []
    for k in range(8):
        b, th = divmod(k, 2)
        in_maps.append(
            {"x_shard": np.ascontiguousarray(x[b, :, 16 * th : 16 * th + 16])}
        )
    return in_maps


def _run(x, trace=False, **spmd_kwargs):
    from concourse.bass_utils import run_bass_kernel_spmd

    nc = _build()
    in_maps = _in_maps(x)

    bkr = run_bass_kernel_spmd(nc, in_maps, list(range(8)), trace=trace, **spmd_kwargs)

    out = np.empty((4, 24, 16, 128, 128), dtype=np.float32)
    for k in range(8):
        b, th = divmod(k, 2)
        out[b, :, 8 * th : 8 * th + 8] = np.asarray(
            bkr.results[k]["y_shard"]
        ).astype(np.float32)
    return out, bkr


def kernel(x):
    out, _ = _run(x)
    return out


# revision 12
# speedup vs baseline: 238.3020x; 1.4954x over previous
"""3D Haar DWT (nn_Patcher) Trainium2 Bass kernel.

Math: with even dims and reflect-pad(0,1) never engaged, the reference is
non-overlapping 2x2x2 Haar butterflies; r^3 * 2*sqrt(2) == 1 exactly, so the
output is pure +/-1 sums over 2x2x2 blocks -- no multiplies needed.

Full input  x  [4, 3, 32, 256, 256] f32
Full output y  [4, 24, 16, 128, 128] f32   (8 subbands x 3 ch on channel dim)

Sharding (8 cores): core k -> (b = k//2, th = k%2); shard input
x[b, :, 16*th : 16*th+16]  -> [3, 16, 256, 256]  (12.58 MB)
shard output -> y[b, :, 8*th : 8*th+8]           -> [24, 8, 128, 128]

Per-core kernel: 6 mega-iters over (c in 3, tg in 2). Partition p = t*32 + h4
(t = output frame in mega-iter 0..3, h4 = h_out//4 0..31). All three Haar
stages stay within-partition. Two deliberate perf choices vs the naive form:

* Output is written in bf16 (error ~1e-3 << the 2e-2 gate) which halves
  output HBM traffic: 12.58+6.29 MB/core = 18.87 MB -> 52.4 us DMA floor
  at 360 GB/s, vs 69.9 us for f32 out.
* Stage order is W -> T -> H, not T -> H -> W. The W butterfly reads
  stride-2 element pairs, which disqualifies the DVE 2x 16-bit mode, so W
  is done FIRST while the data is still f32 (which never gets 2x anyway)
  and converts to bf16 on write. T and H then read packed bf16 at 2x.
  DVE carries 5 of 6 ops (~6.7 us/iter), Pool (gpsimd, 0.42-efficiency
  software adds) carries only W-sub (~4.2 us/iter); both sit under the
  ~8.7 us/iter DMA floor, so steady state is DMA-bound.

DMA: each engine's HWDGE ring serializes its transfers, so input DMAs
alternate SP/Act and output DMAs alternate Act/SP to use both rings and
let outputs start as soon as each mega-iter finishes.
"""

import sys

for _p in ("/opt/trn_rl_repo", "/opt/pypackages"):
    if _p not in sys.path:
        sys.path.append(_p)

import numpy as np

_NC_CACHE = {}


def _build(reps=1, mode="full"):
    # mode: "full" | "dma" (transfers only) | "compute" (engine ops only) —
    # ablations used by calibration to attribute time.
    key = (reps, mode)
    if key in _NC_CACHE:
        return _NC_CACHE[key]

    from concourse import bacc, mybir
    from concourse.tile import TileContext

    fp32 = mybir.dt.float32
    bf16 = mybir.dt.bfloat16
    add = mybir.AluOpType.add
    sub = mybir.AluOpType.subtract

    # Bacc (not plain Bass): its finalize() runs the backend passes, incl.
    # generate_event_semaphores which splits multi-wait instructions (HW
    # allows at most 1 sync wait per instruction).
    nc = bacc.Bacc(None, target_bir_lowering=False)
    x = nc.dram_tensor("x_shard", [3, 16, 256, 256], fp32, kind="ExternalInput")
    y = nc.dram_tensor("y_shard", [24, 8, 128, 128], bf16, kind="ExternalOutput")

    # y viewed as [c, tg, (t h4), s, (rh wh)] ; channels 24 = (s c),
    # s = 4*tb + 2*hb + wb. (t h4) merges to one stride-512 dim matching SBUF
    # partitions; (rh wh) is a contiguous 1 KB run. 3-dim DMA AP both sides.
    yv = y[:].rearrange(
        "(s c) (tg t) (h4 rh) wh -> c tg (t h4) s (rh wh)", s=8, c=3, tg=2, rh=4
    )

    in_cycle = [nc.sync, nc.scalar]
    out_cycle = [nc.scalar, nc.sync]
    ni = 0
    no = 0

    with TileContext(nc) as tc:
        with tc.tile_pool(name="static", bufs=1) as spool:
            # ablation modes read from a static pre-initialized tile so the
            # measured stream carries no false dependencies (pure RAR).
            s_o = s_in = None
            if mode == "dma":
                s_o = spool.tile([128, 8, 512], bf16)
                nc.gpsimd.memset(s_o[:], 0.0)
            elif mode == "compute":
                s_in = spool.tile([128, 2, 8, 256], fp32)
                nc.gpsimd.memset(s_in[:], 0.0)
            with tc.tile_pool(name="pool", bufs=2) as pool:
                for _rep in range(reps):
                    for c in range(3):
                        for tg in range(2):
                            body(nc, tc, pool, x, yv, mode, c, tg,
                                 in_cycle, out_cycle, ni, no, s_o, s_in)
                            if mode != "compute":
                                ni += 2
                                no += 1

                if mode == "compute":
                    # keep y alive with one cheap transfer
                    t_last = pool.tile([128, 8, 512], bf16)
                    nc.vector.memset(t_last[:], 0.0)
                    nc.sync.dma_start(out=yv[0, 0], in_=t_last[:])

    nc.finalize()
    _NC_CACHE[key] = nc
    return nc


def body(nc, tc, pool, x, yv, mode, c, tg, in_cycle, out_cycle, ni, no, s_o, s_in):
    from concourse import mybir

    fp32 = mybir.dt.float32
    bf16 = mybir.dt.bfloat16
    add = mybir.AluOpType.add
    sub = mybir.AluOpType.subtract

    # partition p = (t, h4); free dims per tile:
    # t_in: (f, r, w)          f=frame parity, r=row in blk
    # t_w:  (f, r, wb, wh)     wb=w-band (lo/hi)
    # t_t:  (tb, r, wb, wh)    tb=t-band
    # t_o:  (tb, hb, wb, rh, wh)
    if mode != "compute":
        t_in = pool.tile([128, 2, 8, 256], fp32)
    else:
        t_in = s_in
    if mode != "dma":
        t_w = pool.tile([128, 2, 8, 2, 128], bf16)
        t_t = pool.tile([128, 2, 8, 2, 128], bf16)
        t_o = pool.tile([128, 2, 2, 2, 4, 128], bf16)

    # ---- input DMA: 2 transfers of 1 MB (8 KB runs),
    # split by frame parity so partition (t h4) merges ----
    if mode != "compute":
        f0 = 8 * tg
        for f in range(2):
            src = x[c, f0 + f : f0 + 8 : 2].rearrange(
                "t (h4 r) w -> t h4 (r w)", h4=32
            )
            dst = t_in[:, f].rearrange("p r w -> p (r w)")
            in_cycle[ni % 2].dma_start(out=dst, in_=src)
            ni += 1

    V = nc.vector
    P = nc.gpsimd
    if mode == "dma":
        out_cycle[no % 2].dma_start(out=yv[c, tg], in_=s_o[:])
        return

    # ---- W stage (column parity; f32 in, bf16 out) ----
    t_inv = t_in.rearrange(
        "p f r (wh wl) -> p f r wh wl", wl=2
    )
    V.tensor_tensor(
        out=t_w[:, :, :, 0],
        in0=t_inv[:, :, :, :, 0],
        in1=t_inv[:, :, :, :, 1],
        op=add,
    )
    P.tensor_tensor(
        out=t_w[:, :, :, 1],
        in0=t_inv[:, :, :, :, 0],
        in1=t_inv[:, :, :, :, 1],
        op=sub,
    )

    # ---- T stage (frame parity; bf16 2x on DVE) ----
    V.tensor_tensor(out=t_t[:, 0], in0=t_w[:, 0], in1=t_w[:, 1], op=add)
    V.tensor_tensor(out=t_t[:, 1], in0=t_w[:, 0], in1=t_w[:, 1], op=sub)

    # ---- H stage (row parity; bf16 2x on DVE) ----
    # 4 ops (tb x hb): ISA mem patterns allow at most 3
    # free dims per operand, so tb can't be a 4th dim.
    t_tv = t_t.rearrange("p tb (rh rl) wb wh -> p tb rh rl wb wh", rl=2)
    for tb in range(2):
        for hb, op in ((0, add), (1, sub)):
            V.tensor_tensor(
                out=t_o[:, tb, hb],
                in0=t_tv[:, tb, :, 0].rearrange("p rh wb wh -> p wb rh wh"),
                in1=t_tv[:, tb, :, 1].rearrange("p rh wb wh -> p wb rh wh"),
                op=op,
            )

    # ---- output DMA: 1 transfer of 1 MB (1 KB runs) ----
    if mode != "compute":
        src = t_o.rearrange("p tb hb wb rh wh -> p (tb hb wb) (rh wh)")
        out_cycle[no % 2].dma_start(out=yv[c, tg], in_=src)


def _in_maps(x):
    x = np.ascontiguousarray(np.asarray(x, dtype=np.float32))
    assert x.shape == (4, 3, 32, 256, 256), x.shape
    in_maps = []
    for k in range(8):
        b, th = divmod(k, 2)
        in_maps.append(
            {"x_shard": np.ascontiguousarray(x[b, :, 16 * th : 16 * th + 16])}
        )
    return in_maps


def _run(x, trace=False, **spmd_kwargs):
    from concourse.bass_utils import run_bass_kernel_spmd

    nc = _build()
    in_maps = _in_maps(x)

    bkr = run_bass_kernel_spmd(nc, in_maps, list(range(8)), trace=trace, **spmd_kwargs)

    out = np.empty((4, 24, 16, 128, 128), dtype=np.float32)
    for k in range(8):
        b, th = divmod(k, 2)
        out[b, :, 8 * th : 8 * th + 8] = np.asarray(
            bkr.results[k]["y_shard"]
        ).astype(np.float32)
    return out, bkr


def kernel(x):
    out, _ = _run(x)
    return out


# revision 14
# speedup vs baseline: 393.1238x; 1.6497x over previous
"""3D Haar DWT (nn_Patcher) Trainium2 Bass kernel.

Math: with even dims the reflect-pad(0,1) never engages, so the reference is
non-overlapping 2x2x2 Haar butterflies; r^3 * 2*sqrt(2) == 1 exactly, so the
output is pure +/-1 sums over 2x2x2 blocks -- no multiplies needed.

Full input  x  [4, 3, 32, 256, 256] f32
Full output y  [4, 24, 16, 128, 128] f32   (8 subbands x 3 ch on channel dim)

Sharding (8 cores): core k -> (b = k//2, th = k%2); input shard
x[b, :, 16*th : 16*th+16] -> [3, 16, 256, 256] (12.58 MB); output shard
y[b, :, 8*th : 8*th+8] -> [24, 8, 128, 128]. Pure data parallel, no
cross-core communication.

Per-core kernel: 3 mega-iters over c. Partition p = t*16 + h8 (t = output
frame 0..7, h8 = h_out//8 0..15); all three Haar stages stay
within-partition. Perf-relevant choices:

* Few, large DMA transfers: 6x 2MB in (16KB runs) + 3x 2MB out (2KB runs)
  per rep. DMA here costs a fixed per-transfer overhead plus a per-byte
  stream cost, so transfer count matters as much as bytes.
* bf16 output (error ~3e-3 << the 2e-2 gate) halves output HBM traffic;
  the host upcasts to f32.
* Stage order W -> T -> H: the W butterfly reads stride-2 element pairs,
  which disqualifies the DVE 2x 16-bit mode, so W runs FIRST while the
  data is still f32 (which never gets 2x anyway) and converts to bf16 on
  write; T and H then process packed bf16 at 2x. W-sub runs on Pool
  (gpsimd), the other five ops on DVE; both sit well under the DMA stream.
* t_in is triple-buffered so input transfers stay queued ahead of compute;
  the H stage is 4 ops (tb x hb) because operand APs allow <=3 free dims.
"""

import sys

for _p in ("/opt/trn_rl_repo", "/opt/pypackages"):
    if _p not in sys.path:
        sys.path.append(_p)

import numpy as np

_NC_CACHE = {}

IN_QS = ("sync", "scalar")  # input DMA rings, alternating
OUT_QS = ("scalar", "sync")  # output DMA rings, alternating
BUFS = 2


def _build(reps=1, mode="full"):
    key = (reps, mode)
    if key in _NC_CACHE:
        return _NC_CACHE[key]

    from concourse import bacc, mybir
    from concourse.tile import TileContext

    fp32 = mybir.dt.float32
    bf16 = mybir.dt.bfloat16
    add = mybir.AluOpType.add
    sub = mybir.AluOpType.subtract

    nc = bacc.Bacc(None, target_bir_lowering=False)
    x = nc.dram_tensor("x_shard", [3, 16, 256, 256], fp32, kind="ExternalInput")
    y = nc.dram_tensor("y_shard", [24, 8, 128, 128], bf16, kind="ExternalOutput")

    # y viewed as [c, (t h8), s, (rh wh)]; s = 4*tb + 2*hb + wb; (rh wh) is a
    # contiguous 2KB run (8 output rows x 128 w).
    yv = y[:].rearrange("(s c) t (h8 rh) wh -> c (t h8) s (rh wh)", s=8, c=3, rh=8)

    in_cycle = [getattr(nc, q) for q in IN_QS]
    out_cycle = [getattr(nc, q) for q in OUT_QS]
    ni = 0
    no = 0

    with TileContext(nc) as tc:
        with tc.tile_pool(name="static", bufs=1) as spool:
            s_o = s_in = None
            if mode == "dma":
                s_o = spool.tile([128, 8, 1024], bf16)
                nc.gpsimd.memset(s_o[:], 0.0)
            elif mode == "compute":
                s_in = spool.tile([128, 2, 16, 256], fp32)
                nc.gpsimd.memset(s_in[:], 0.0)
            with tc.tile_pool(name="pool", bufs=BUFS) as pool:
                for _rep in range(reps):
                    for c in range(3):
                        _body(nc, pool, x, yv, mode, c,
                              in_cycle[ni % 2], in_cycle[(ni + 1) % 2],
                              out_cycle[no % 2], s_o, s_in)
                        if mode != "compute":
                            ni += 2
                            no += 1

                if mode == "compute":
                    t_last = pool.tile([128, 8, 1024], bf16)
                    nc.vector.memset(t_last[:], 0.0)
                    nc.sync.dma_start(out=yv[0], in_=t_last[:])

    nc.finalize()
    _NC_CACHE[key] = nc
    return nc


def _body(nc, pool, x, yv, mode, c, in_q0, in_q1, out_q, s_o, s_in):
    from concourse import mybir

    fp32 = mybir.dt.float32
    bf16 = mybir.dt.bfloat16
    add = mybir.AluOpType.add
    sub = mybir.AluOpType.subtract

    # partition p = (t, h8); free dims per tile:
    # t_in: (f, r, w)       f = frame parity, r = input row in block (16)
    # t_w:  (f, r, wb, wh)
    # t_t:  (tb, r, wb, wh)
    # t_o:  (tb, hb, wb, rh, wh)   rh = output row in block (8)
    if mode != "compute":
        t_in = pool.tile([128, 2, 16, 256], fp32, bufs=3)
    else:
        t_in = s_in
    if mode != "dma":
        t_w = pool.tile([128, 2, 16, 2, 128], bf16)
        t_t = pool.tile([128, 2, 16, 2, 128], bf16)
        t_o = pool.tile([128, 2, 2, 2, 8, 128], bf16)

    # ---- input DMA: 2 transfers of 2 MB (16 KB runs) ----
    if mode != "compute":
        for f, q in ((0, in_q0), (1, in_q1)):
            src = x[c, f::2].rearrange("t (h8 r) w -> t h8 (r w)", h8=16)
            dst = t_in[:, f].rearrange("p r w -> p (r w)")
            q.dma_start(out=dst, in_=src)

    V = nc.vector
    P = nc.gpsimd
    if mode == "dma":
        out_q.dma_start(out=yv[c], in_=s_o[:])
        return

    # ---- W stage (column parity; f32 in, bf16 out) ----
    t_inv = t_in.rearrange("p f r (wh wl) -> p f r wh wl", wl=2)
    V.tensor_tensor(
        out=t_w[:, :, :, 0],
        in0=t_inv[:, :, :, :, 0],
        in1=t_inv[:, :, :, :, 1],
        op=add,
    )
    P.tensor_tensor(
        out=t_w[:, :, :, 1],
        in0=t_inv[:, :, :, :, 0],
        in1=t_inv[:, :, :, :, 1],
        op=sub,
    )

    # ---- T stage (frame parity; bf16 2x on DVE) ----
    V.tensor_tensor(out=t_t[:, 0], in0=t_w[:, 0], in1=t_w[:, 1], op=add)
    V.tensor_tensor(out=t_t[:, 1], in0=t_w[:, 0], in1=t_w[:, 1], op=sub)

    # ---- H stage (row parity; bf16 2x on DVE; <=3 free dims per AP) ----
    t_tv = t_t.rearrange("p tb (rh rl) wb wh -> p tb rh rl wb wh", rl=2)
    for tb in range(2):
        for hb, op in ((0, add), (1, sub)):
            V.tensor_tensor(
                out=t_o[:, tb, hb],
                in0=t_tv[:, tb, :, 0].rearrange("p rh wb wh -> p wb rh wh"),
                in1=t_tv[:, tb, :, 1].rearrange("p rh wb wh -> p wb rh wh"),
                op=op,
            )

    # ---- output DMA: 1 transfer of 2 MB (2 KB runs) ----
    if mode != "compute":
        src = t_o.rearrange("p tb hb wb rh wh -> p (tb hb wb) (rh wh)")
        out_q.dma_start(out=yv[c], in_=src)


def _in_maps(x):
    x = np.ascontiguousarray(np.asarray(x, dtype=np.float32))
    assert x.shape == (4, 3, 32, 256, 256), x.shape
    in_maps = []
    for k in range(8):
        b, th = divmod(k, 2)
        in_maps.append(
            {"x_shard": np.ascontiguousarray(x[b, :, 16 * th : 16 * th + 16])}
        )
    return in_maps


def _run(x, trace=False, **spmd_kwargs):
    from concourse.bass_utils import run_bass_kernel_spmd

    nc = _build()
    in_maps = _in_maps(x)

    bkr = run_bass_kernel_spmd(nc, in_maps, list(range(8)), trace=trace, **spmd_kwargs)

    out = np.empty((4, 24, 16, 128, 128), dtype=np.float32)
    for k in range(8):
        b, th = divmod(k, 2)
        out[b, :, 8 * th : 8 * th + 8] = np.asarray(
            bkr.results[k]["y_shard"]
        ).astype(np.float32)
    return out, bkr


def kernel(x):
    out, _ = _run(x)
    return out


# revision 15
# speedup vs baseline: 404.1986x; 1.0282x over previous
"""3D Haar DWT (nn_Patcher) Trainium2 Bass kernel (PE-butterfly design).

Input DMA is the dominant cost on this device (fixed per-transfer overhead +
per-byte stream, globally serialized), so the kernel is structured to allow 4MB input
transfers with 32KB descriptors: partition p = (tf, hb) = input frame (16) x
32-row block (8). That makes the input per partition one contiguous 32KB run
(3-dim AP), but splits frame pairs across partitions, so the T-stage
butterfly runs on the PE systolic array: out = W.T @ t_w with W a 128x128
+/-1 pairing matrix (supplied as a tiny extra input), accumulated in PSUM
and drained to SBUF as bf16 by the otherwise-idle Act engine.

Stages: W (DVE+Pool, f32 strided -> bf16), T (PE, 16x 512-col matmuls per c,
Act drains), H (DVE, 2 bf16 ops). Output: 2 transfers per c (one per
t-band), 4KB runs.

Per-core shards: x[b,:,16*th:16*th+16], y[b,:,8*th:8*th+8].
"""

import sys

for _p in ("/opt/trn_rl_repo", "/opt/pypackages"):
    if _p not in sys.path:
        sys.path.append(_p)

import numpy as np

_NC_CACHE = {}


def _wmat():
    # W[k, m]: k = tf*8 + hb (input frame, row-block), m = tb*64 + t*8 + hb.
    # T-lo (tb=0) = even + odd frame; T-hi (tb=1) = even - odd frame.
    import ml_dtypes

    w = np.zeros((128, 128), dtype=np.float32)
    for k in range(128):
        tf, hb = divmod(k, 8)
        t, par = divmod(tf, 2)
        w[k, 0 * 64 + t * 8 + hb] = 1.0
        w[k, 1 * 64 + t * 8 + hb] = 1.0 if par == 0 else -1.0
    return w.astype(ml_dtypes.bfloat16)


def _build(reps=1):
    if reps in _NC_CACHE:
        return _NC_CACHE[reps]

    from concourse import bacc, mybir
    from concourse.tile import TileContext

    fp32 = mybir.dt.float32
    bf16 = mybir.dt.bfloat16
    add = mybir.AluOpType.add
    sub = mybir.AluOpType.subtract
    copy_f = mybir.ActivationFunctionType.Copy

    nc = bacc.Bacc(None, target_bir_lowering=False)
    x = nc.dram_tensor("x_shard", [3, 16, 256, 256], fp32, kind="ExternalInput")
    wd = nc.dram_tensor("w_butterfly", [128, 128], bf16, kind="ExternalInput")
    y = nc.dram_tensor("y_shard", [24, 8, 128, 128], bf16, kind="ExternalOutput")

    # dst view: [c, tb, (t hb)=64 partitions, (hbb wb)=4, (rh wh)=4KB run]
    yv = y[:].rearrange(
        "(tb hbb wb c) t (hb rh) wh -> c tb (t hb) (hbb wb) (rh wh)",
        tb=2, hbb=2, wb=2, c=3, rh=16,
    )

    with TileContext(nc) as tc:
        with tc.tile_pool(name="static", bufs=1) as spool:
            t_wm = spool.tile([128, 128], bf16)
            nc.sync.dma_start(out=t_wm[:], in_=wd[:])
            with tc.tile_pool(name="pool", bufs=2) as pool:
                with tc.psum_pool(name="psum", bufs=2) as ppool:
                    for _rep in range(reps):
                        for c in range(3):
                            _body(nc, pool, ppool, x, yv, c, t_wm,
                                  fp32, bf16, add, sub, copy_f)

    nc.finalize()
    _NC_CACHE[reps] = nc
    return nc


def _body(nc, pool, ppool, x, yv, c, t_wm, fp32, bf16, add, sub, copy_f):
    # partition p = (tf, hb): tf = input frame (16), hb = 32-row block (8)
    # t_in: (r32, w256) f32        t_w: (r32, wb2, wh128) bf16
    # partition q = (tb, t, hb) after PE:
    # t_t: (r32, wb2, wh128) bf16  t_o: (hbb2, wb2, rh16, wh128) bf16
    t_in = pool.tile([128, 32, 256], fp32, bufs=3)
    t_w = pool.tile([128, 32, 2, 128], bf16)
    t_t = pool.tile([128, 32, 2, 128], bf16)
    t_o = pool.tile([128, 2, 2, 16, 128], bf16)

    # ---- input DMA: ONE transfer of 4 MB (32 KB runs) ----
    src = x[c].rearrange("tf (hb r) w -> tf hb (r w)", hb=8)
    nc.sync.dma_start(out=t_in.rearrange("p r w -> p (r w)"), in_=src)

    V = nc.vector
    P = nc.gpsimd

    # ---- W stage (column parity; f32 in, bf16 out) ----
    t_inv = t_in.rearrange("p r (wh wl) -> p r wh wl", wl=2)
    V.tensor_tensor(
        out=t_w[:, :, 0], in0=t_inv[:, :, :, 0], in1=t_inv[:, :, :, 1], op=add
    )
    P.tensor_tensor(
        out=t_w[:, :, 1], in0=t_inv[:, :, :, 0], in1=t_inv[:, :, :, 1], op=sub
    )

    # ---- T stage on PE: t_t[q] = sum_p W[p, q] * t_w[p] ----
    t_wv = t_w.rearrange("p r wb wh -> p (r wb wh)").rearrange(
        "p (ch n) -> p ch n", n=512
    )
    t_tv = t_t.rearrange("p r wb wh -> p (r wb wh)").rearrange(
        "p (ch n) -> p ch n", n=512
    )
    for ch in range(16):
        t_ps = ppool.tile([128, 512], fp32, name="t_ps")
        nc.tensor.matmul(
            out=t_ps[:], lhsT=t_wm[:], rhs=t_wv[:, ch], start=True, stop=True
        )
        nc.scalar.activation(out=t_tv[:, ch], in_=t_ps[:], func=copy_f)

    # ---- H stage (row parity; bf16 2x on DVE) ----
    t_th = t_t.rearrange("p (rh rl) wb wh -> p rh rl wb wh", rl=2)
    for hbb, op in ((0, add), (1, sub)):
        V.tensor_tensor(
            out=t_o[:, hbb],
            in0=t_th[:, :, 0].rearrange("p rh wb wh -> p wb rh wh"),
            in1=t_th[:, :, 1].rearrange("p rh wb wh -> p wb rh wh"),
            op=op,
        )

    # ---- output DMA: 2 transfers of 1 MB (4 KB runs), one per t-band ----
    for tb, q in ((0, nc.scalar), (1, nc.scalar)):
        src = t_o[64 * tb : 64 * tb + 64].rearrange(
            "p hbb wb rh wh -> p (hbb wb) (rh wh)"
        )
        q.dma_start(out=yv[c, tb], in_=src)


def _in_maps(x):
    x = np.ascontiguousarray(np.asarray(x, dtype=np.float32))
    assert x.shape == (4, 3, 32, 256, 256), x.shape
    w = _wmat()
    in_maps = []
    for k in range(8):
        b, th = divmod(k, 2)
        in_maps.append(
            {
                "x_shard": np.ascontiguousarray(x[b, :, 16 * th : 16 * th + 16]),
                "w_butterfly": w,
            }
        )
    return in_maps


def _run(x, trace=False, **spmd_kwargs):
    from concourse.bass_utils import run_bass_kernel_spmd

    nc = _build()
    in_maps = _in_maps(x)

    bkr = run_bass_kernel_spmd(nc, in_maps, list(range(8)), trace=trace, **spmd_kwargs)

    out = np.empty((4, 24, 16, 128, 128), dtype=np.float32)
    for k in range(8):
        b, th = divmod(k, 2)
        out[b, :, 8 * th : 8 * th + 8] = np.asarray(
            bkr.results[k]["y_shard"]
        ).astype(np.float32)
    return out, bkr


def kernel(x):
    out, _ = _run(x)
    return out
